# revision 2
# baseline (speedup 1.0000x reference)
"""AdditiveAttention Trainium2 kernel (Bass/Tile), 8-core data-parallel.

Math (per batch b):
    q = queries @ Wq.T              [Q, H]
    k = keys @ Wk.T                 [K, H]
    scores[q,k] = sum_h Wv[h] * tanh(q[q,h] + k[k,h])
    attn = softmax(mask(scores))    masked positions -> -1e6
    out = attn @ values             [Q, V]

Device mapping (per core, 2 batches/core):
  - tanh stage on ScalarE: per (b, qi, h-tile) one ACT instr
    tanh(kT[h,ki] + bias=qT[h,qi]) on [128, 512]; broadcast-add fused as
    the per-partition activation bias.  This is the roofline engine.
  - score reduction on PE: tanh tile is the stationary operand
    (lhsT [h,128 ki]), rhs = Wv column [h,1]; accumulates over the two
    h-tiles into PSUM column (ki, qi) -> scores^T [ki, qi] per k-block.
  - exp on ScalarE straight out of PSUM with bias = mask column
    (0 / -1e6 precomputed on host from valid_lens); no max-subtraction
    (scores are O(1) by construction; masked lanes underflow to 0).
  - AV on PE: lhsT = exp tile [ki, qi], rhs = [values | ones] [ki, 257];
    PSUM [qi, 257] accumulates over k-blocks, last column = softmax
    denominator.  Reciprocal + per-partition scale on VectorE.
"""

import numpy as np

import jax
from jax.sharding import Mesh, PartitionSpec
from jax.experimental.shard_map import shard_map

import concourse.bass as bass
import concourse.mybir as mybir
import concourse.tile as tile
from concourse import bacc, bass2jax
from concourse.masks import make_identity

B, Q, K, H, V = 16, 128, 512, 256, 256
N_CORES = 8
B_LOC = B // N_CORES  # 2 batches per core
P = 128
HT = H // P   # 2 h-tiles
DT = H // P   # 2 d-tiles (projection contraction)
KB = K // P   # 4 k-blocks
F32 = mybir.dt.float32
MASK_VAL = -1e6


def _emit(nc, tc, queries_d, keys_d, values_d, wq_d, wk_d, wv_d, mask_d, out_d, ctx):
    const = ctx.enter_context(tc.tile_pool(name="const", bufs=1))
    stage = ctx.enter_context(tc.tile_pool(name="stage", bufs=2))
    persist = ctx.enter_context(tc.tile_pool(name="persist", bufs=1))
    twork = ctx.enter_context(tc.tile_pool(name="twork", bufs=6))
    ps_misc = ctx.enter_context(tc.tile_pool(name="ps_misc", bufs=2, space="PSUM"))
    ps_sc = ctx.enter_context(tc.tile_pool(name="ps_sc", bufs=2, space="PSUM"))
    ps_out = ctx.enter_context(tc.tile_pool(name="ps_out", bufs=2, space="PSUM"))

    ident = const.tile([P, P], F32)
    make_identity(nc, ident)

    # --- weights: load natural [h, d], PE-transpose into [d, h] ---
    wq_nat = const.tile([P, HT, H], F32)
    nc.sync.dma_start(out=wq_nat, in_=wq_d.rearrange("(t p) d -> p t d", p=P))
    wk_nat = const.tile([P, HT, H], F32)
    nc.sync.dma_start(out=wk_nat, in_=wk_d.rearrange("(t p) d -> p t d", p=P))
    wqT = const.tile([P, DT, H], F32)  # [d_in, dt, h]
    wkT = const.tile([P, DT, H], F32)
    for (w_nat, w_T) in ((wq_nat, wqT), (wk_nat, wkT)):
        for ht in range(HT):
            for dt in range(DT):
                ps = ps_misc.tile([P, P], F32, tag="ps_tr")
                nc.tensor.transpose(ps, w_nat[:, ht, dt * P:(dt + 1) * P], ident)
                nc.vector.tensor_copy(out=w_T[:, dt, ht * P:(ht + 1) * P], in_=ps)

    wv_sb = const.tile([P, HT], F32)
    nc.sync.dma_start(out=wv_sb, in_=wv_d.rearrange("(t p) -> p t", p=P))
    mask_sb = const.tile([P, B_LOC, KB], F32)
    nc.sync.dma_start(out=mask_sb, in_=mask_d.rearrange("b (kb p) -> p b kb", p=P))

    for b in range(B_LOC):
        # --- load + transpose queries/keys, project ---
        q_nat = stage.tile([P, H], F32, tag="qnat")
        nc.sync.dma_start(out=q_nat, in_=queries_d[b])
        qTd = stage.tile([P, DT, Q], F32, tag="qTd")  # [d_in, dt, qi]
        for dt in range(DT):
            ps = ps_misc.tile([P, P], F32, tag="ps_tr")
            nc.tensor.transpose(ps, q_nat[:, dt * P:(dt + 1) * P], ident)
            nc.vector.tensor_copy(out=qTd[:, dt, :], in_=ps)

        k_nat = stage.tile([P, KB, H], F32, tag="knat")
        nc.sync.dma_start(out=k_nat, in_=keys_d[b].rearrange("(kt p) d -> p kt d", p=P))
        kTd = stage.tile([P, DT, K], F32, tag="kTd")  # [d_in, dt, ki]
        for kt in range(KB):
            for dt in range(DT):
                ps = ps_misc.tile([P, P], F32, tag="ps_tr")
                nc.tensor.transpose(ps, k_nat[:, kt, dt * P:(dt + 1) * P], ident)
                nc.vector.tensor_copy(out=kTd[:, dt, kt * P:(kt + 1) * P], in_=ps)

        qT = persist.tile([P, HT, Q], F32, tag=f"qT{b}")  # [h_in, ht, qi]
        for ht in range(HT):
            ps = ps_misc.tile([P, P], F32, tag="ps_tr")
            for dt in range(DT):
                nc.tensor.matmul(ps, wqT[:, dt, ht * P:(ht + 1) * P], qTd[:, dt, :],
                                 start=(dt == 0), stop=(dt == DT - 1))
            nc.vector.tensor_copy(out=qT[:, ht, :], in_=ps)

        kT = persist.tile([P, HT, K], F32, tag=f"kT{b}")  # [h_in, ht, ki]
        for ht in range(HT):
            ps = ps_misc.tile([P, K], F32, tag="ps_prj")
            for dt in range(DT):
                nc.tensor.matmul(ps, wkT[:, dt, ht * P:(ht + 1) * P], kTd[:, dt, :],
                                 start=(dt == 0), stop=(dt == DT - 1))
            nc.vector.tensor_copy(out=kT[:, ht, :], in_=ps)

        # --- values with appended ones column ---
        vo = []
        for kb in range(KB):
            t = persist.tile([P, V + 1], F32, tag=f"vo{b}{kb}")
            nc.sync.dma_start(out=t[:, :V], in_=values_d[b, kb * P:(kb + 1) * P, :])
            nc.vector.memset(t[:, V:V + 1], 1.0)
            vo.append(t)

        # --- main loop: tanh + score columns ---
        # psum scores^T: [ki_in, kb*128 + qi]
        sc = ps_sc.tile([P, K], F32, tag="sc")
        for qi in range(Q):
            tt = []
            for ht in range(HT):
                t = twork.tile([P, K], F32, tag="T")
                nc.scalar.activation(out=t, in_=kT[:, ht, :],
                                     func=mybir.ActivationFunctionType.Tanh,
                                     bias=qT[:, ht, qi:qi + 1])
                tt.append(t)
            for kb in range(KB):
                col = kb * P + qi
                for ht in range(HT):
                    nc.tensor.matmul(sc[:, col:col + 1],
                                     tt[ht][:, kb * P:(kb + 1) * P],
                                     wv_sb[:, ht:ht + 1],
                                     start=(ht == 0), stop=(ht == HT - 1))

        # --- exp (+mask) and AV ---
        po = ps_out.tile([P, V + 1], F32, tag="po")
        for kb in range(KB):
            e = persist.tile([P, Q], F32, tag=f"E{b}{kb}")
            nc.scalar.activation(out=e, in_=sc[:, kb * P:(kb + 1) * P],
                                 func=mybir.ActivationFunctionType.Exp,
                                 bias=mask_sb[:, b, kb:kb + 1])
            nc.tensor.matmul(po, e, vo[kb], start=(kb == 0), stop=(kb == KB - 1))

        r = stage.tile([P, 1], F32, tag="recip")
        nc.vector.reciprocal(out=r, in_=po[:, V:V + 1])
        ot = stage.tile([P, V], F32, tag="ot")
        nc.vector.tensor_scalar_mul(ot, po[:, :V], r)
        nc.sync.dma_start(out=out_d[b], in_=ot)


def build_nc(repeat=1):
    from contextlib import ExitStack
    nc = bacc.Bacc("TRN2", target_bir_lowering=False, debug=False,
                   num_devices=N_CORES)
    queries_d = nc.dram_tensor("queries", [B_LOC, Q, H], F32, kind="ExternalInput").ap()
    keys_d = nc.dram_tensor("keys", [B_LOC, K, H], F32, kind="ExternalInput").ap()
    values_d = nc.dram_tensor("values", [B_LOC, K, V], F32, kind="ExternalInput").ap()
    wq_d = nc.dram_tensor("Wq", [H, H], F32, kind="ExternalInput").ap()
    wk_d = nc.dram_tensor("Wk", [H, H], F32, kind="ExternalInput").ap()
    wv_d = nc.dram_tensor("Wv", [H], F32, kind="ExternalInput").ap()
    mask_d = nc.dram_tensor("mask", [B_LOC, K], F32, kind="ExternalInput").ap()
    out_d = nc.dram_tensor("out", [B_LOC, Q, V], F32, kind="ExternalOutput").ap()

    with tile.TileContext(nc) as tc, ExitStack() as ctx:
        args = (nc, tc, queries_d, keys_d, values_d, wq_d, wk_d, wv_d, mask_d, out_d, ctx)
        if repeat == 1:
            _emit(*args)
        else:
            with tc.For_i(0, repeat, 1):
                _emit(*args)
    nc.compile()
    return nc


def make_runner(nc):
    """Replicates bass2jax.run_bass_via_pjrt but jits once for reuse."""
    bass2jax.install_neuronx_cc_hook()
    partition_name = nc.partition_id_tensor.name if nc.partition_id_tensor else None
    in_names, out_names, out_avals, zero_shapes = [], [], [], []
    for alloc in nc.m.functions[0].allocations:
        if not isinstance(alloc, mybir.MemoryLocationSet):
            continue
        name = alloc.memorylocations[0].name
        if alloc.kind == "ExternalInput":
            if name != partition_name:
                in_names.append(name)
        elif alloc.kind == "ExternalOutput":
            shape = tuple(alloc.tensor_shape)
            npdt = np.dtype(mybir.dt.np(alloc.dtype))
            out_names.append(name)
            out_avals.append(jax.core.ShapedArray(shape, npdt))
            zero_shapes.append((shape, npdt))
    n_params = len(in_names)
    n_outs = len(out_avals)
    in_names_all = list(in_names) + list(out_names)
    if partition_name is not None:
        in_names_all.append(partition_name)

    def _body(*args):
        operands = list(args)
        if partition_name is not None:
            operands.append(bass2jax.partition_id_tensor())
        outs = bass2jax._bass_exec_p.bind(
            *operands,
            out_avals=tuple(out_avals),
            in_names=tuple(in_names_all),
            out_names=tuple(out_names),
            lowering_input_output_aliases=(),
            sim_require_finite=True,
            sim_require_nnan=True,
            nc=nc,
        )
        return tuple(outs)

    devices = jax.devices()[:N_CORES]
    mesh = Mesh(np.asarray(devices), ("core",))
    in_specs = (PartitionSpec("core"),) * (n_params + n_outs)
    out_specs = (PartitionSpec("core"),) * len(out_names)
    sharded = jax.jit(
        shard_map(_body, mesh=mesh, in_specs=in_specs, out_specs=out_specs,
                  check_rep=False),
        donate_argnums=tuple(range(n_params, n_params + n_outs)),
        keep_unused=True,
    )

    def run(in_maps):
        per_core = [[np.asarray(m[name]) for name in in_names] for m in in_maps]
        concat_in = [np.concatenate([per_core[c][i] for c in range(N_CORES)], axis=0)
                     for i in range(n_params)]
        concat_zeros = [np.zeros((N_CORES * s[0], *s[1:]), d) for (s, d) in zero_shapes]
        out_arrs = sharded(*concat_in, *concat_zeros)
        out_arrs = [np.asarray(a) for a in out_arrs]
        return [
            {name: out_arrs[i].reshape(N_CORES, *out_avals[i].shape)[c]
             for i, name in enumerate(out_names)}
            for c in range(N_CORES)
        ]

    return run


_RUNNERS = {}


def get_runner(repeat=1):
    if repeat not in _RUNNERS:
        _RUNNERS[repeat] = make_runner(build_nc(repeat))
    return _RUNNERS[repeat]


def make_in_maps(queries, keys, values, Wq, Wk, Wv, valid_lens):
    queries = np.asarray(queries, np.float32)
    keys = np.asarray(keys, np.float32)
    values = np.asarray(values, np.float32)
    Wq = np.asarray(Wq, np.float32)
    Wk = np.asarray(Wk, np.float32)
    Wv = np.asarray(Wv, np.float32)
    valid_lens = np.asarray(valid_lens)
    mask = np.where(np.arange(K)[None, :] < valid_lens[:, None].astype(np.int64),
                    0.0, MASK_VAL).astype(np.float32)
    in_maps = []
    for c in range(N_CORES):
        sl = slice(c * B_LOC, (c + 1) * B_LOC)
        in_maps.append({
            "queries": queries[sl], "keys": keys[sl], "values": values[sl],
            "Wq": Wq, "Wk": Wk, "Wv": Wv, "mask": mask[sl],
        })
    return in_maps


def kernel(queries, keys, values, Wq, Wk, Wv, valid_lens):
    run = get_runner(1)
    in_maps = make_in_maps(queries, keys, values, Wq, Wk, Wv, valid_lens)
    res = run(in_maps)
    return np.concatenate([res[c]["out"] for c in range(N_CORES)], axis=0)


# revision 5
# speedup vs baseline: 4.6274x; 4.6274x over previous
"""AdditiveAttention Trainium2 kernel (Bass/Tile), 8-core data-parallel.

Math (per batch b):
    q = queries @ Wq.T              [Q, H]
    k = keys @ Wk.T                 [K, H]
    scores[q,k] = sum_h Wv[h] * tanh(q[q,h] + k[k,h])
    attn = softmax(mask(scores))    masked positions -> -1e6
    out = attn @ values             [Q, V]

Device mapping (per core, 2 batches/core):
  - tanh stage on ScalarE: per (b, qi, h-tile) one ACT instr
    tanh(kT[h,ki] + bias=qT[h,qi]) on [128, 512]; broadcast-add fused as
    the per-partition activation bias.  This is the roofline engine.
  - score reduction on PE: tanh tile is the stationary operand
    (lhsT [h,128 ki]), rhs = Wv column [h,1]; accumulates over the two
    h-tiles into PSUM column (ki, qi) -> scores^T [ki, qi] per k-block.
  - exp on ScalarE straight out of PSUM with bias = mask column
    (0 / -1e6 precomputed on host from valid_lens); no max-subtraction
    (scores are O(1) by construction; masked lanes underflow to 0).
  - AV on PE: lhsT = exp tile [ki, qi], rhs = [values | ones] [ki, 257];
    PSUM [qi, 257] accumulates over k-blocks, last column = softmax
    denominator.  Reciprocal + per-partition scale on VectorE.
"""

import numpy as np

import jax
from jax.sharding import Mesh, PartitionSpec
from jax.experimental.shard_map import shard_map

import concourse.bass as bass
import concourse.mybir as mybir
import concourse.tile as tile
from concourse import bacc, bass2jax
from concourse.masks import make_identity

B, Q, K, H, V = 16, 128, 512, 256, 256
N_CORES = 8
B_LOC = B // N_CORES  # 2 batches per core
P = 128
HT = H // P   # 2 h-tiles
DT = H // P   # 2 d-tiles (projection contraction)
KB = K // P   # 4 k-blocks
F32 = mybir.dt.float32
F16 = mybir.dt.float16
MASK_VAL = -1e6


def _emit(nc, tc, queries_d, keys_d, values_d, wq_d, wk_d, wv_d, mask_d, out_d, ctx):
    const = ctx.enter_context(tc.tile_pool(name="const", bufs=1))
    stage = ctx.enter_context(tc.tile_pool(name="stage", bufs=2))
    persist = ctx.enter_context(tc.tile_pool(name="persist", bufs=1))
    twork = ctx.enter_context(tc.tile_pool(name="twork", bufs=6))
    ps_misc = ctx.enter_context(tc.tile_pool(name="ps_misc", bufs=2, space="PSUM"))
    ps_sc = ctx.enter_context(tc.tile_pool(name="ps_sc", bufs=2, space="PSUM"))
    ps_out = ctx.enter_context(tc.tile_pool(name="ps_out", bufs=2, space="PSUM"))

    ident = const.tile([P, P], F32)
    make_identity(nc, ident)

    # --- weights: load natural [h, d], PE-transpose into [d, h] ---
    wq_nat = const.tile([P, HT, H], F32)
    nc.sync.dma_start(out=wq_nat, in_=wq_d.rearrange("(t p) d -> p t d", p=P))
    wk_nat = const.tile([P, HT, H], F32)
    nc.sync.dma_start(out=wk_nat, in_=wk_d.rearrange("(t p) d -> p t d", p=P))
    wqT = const.tile([P, DT, H], F32)  # [d_in, dt, h]
    wkT = const.tile([P, DT, H], F32)
    for (w_nat, w_T) in ((wq_nat, wqT), (wk_nat, wkT)):
        for ht in range(HT):
            for dt in range(DT):
                ps = ps_misc.tile([P, P], F32, tag="ps_tr")
                nc.tensor.transpose(ps, w_nat[:, ht, dt * P:(dt + 1) * P], ident)
                nc.vector.tensor_copy(out=w_T[:, dt, ht * P:(ht + 1) * P], in_=ps)

    wv_sb = const.tile([P, HT], F16)
    nc.gpsimd.dma_start(out=wv_sb, in_=wv_d.rearrange("(t p) -> p t", p=P))
    mask_sb = const.tile([P, B_LOC, KB], F32)
    nc.sync.dma_start(out=mask_sb, in_=mask_d.rearrange("b (kb p) -> p b kb", p=P))

    for b in range(B_LOC):
        # --- load + transpose queries/keys, project ---
        q_nat = stage.tile([P, H], F32, tag="qnat")
        nc.sync.dma_start(out=q_nat, in_=queries_d[b])
        qTd = stage.tile([P, DT, Q], F32, tag="qTd")  # [d_in, dt, qi]
        for dt in range(DT):
            ps = ps_misc.tile([P, P], F32, tag="ps_tr")
            nc.tensor.transpose(ps, q_nat[:, dt * P:(dt + 1) * P], ident)
            nc.vector.tensor_copy(out=qTd[:, dt, :], in_=ps)

        k_nat = stage.tile([P, KB, H], F32, tag="knat")
        nc.sync.dma_start(out=k_nat, in_=keys_d[b].rearrange("(kt p) d -> p kt d", p=P))
        kTd = stage.tile([P, DT, K], F32, tag="kTd")  # [d_in, dt, ki]
        for kt in range(KB):
            for dt in range(DT):
                ps = ps_misc.tile([P, P], F32, tag="ps_tr")
                nc.tensor.transpose(ps, k_nat[:, kt, dt * P:(dt + 1) * P], ident)
                nc.vector.tensor_copy(out=kTd[:, dt, kt * P:(kt + 1) * P], in_=ps)

        qT = persist.tile([P, HT, Q], F32, tag=f"qT{b}")  # [h_in, ht, qi]
        for ht in range(HT):
            ps = ps_misc.tile([P, P], F32, tag="ps_tr")
            for dt in range(DT):
                nc.tensor.matmul(ps, wqT[:, dt, ht * P:(ht + 1) * P], qTd[:, dt, :],
                                 start=(dt == 0), stop=(dt == DT - 1))
            nc.vector.tensor_copy(out=qT[:, ht, :], in_=ps)

        kT = persist.tile([P, HT, K], F32, tag=f"kT{b}")  # [h_in, ht, ki]
        for ht in range(HT):
            ps = ps_misc.tile([P, K], F32, tag="ps_prj")
            for dt in range(DT):
                nc.tensor.matmul(ps, wkT[:, dt, ht * P:(ht + 1) * P], kTd[:, dt, :],
                                 start=(dt == 0), stop=(dt == DT - 1))
            nc.vector.tensor_copy(out=kT[:, ht, :], in_=ps)

        # --- values with appended ones column ---
        vo = []
        for kb in range(KB):
            t = persist.tile([P, V + 1], F32, tag=f"vo{b}{kb}")
            nc.sync.dma_start(out=t[:, :V], in_=values_d[b, kb * P:(kb + 1) * P, :])
            nc.vector.memset(t[:, V:V + 1], 1.0)
            vo.append(t)

        # --- main loop: tanh + score columns ---
        # psum scores^T: [ki_in, kb*128 + qi]
        sc = ps_sc.tile([P, K], F32, tag="sc")
        for qi in range(Q):
            tt = []
            for ht in range(HT):
                t = twork.tile([P, K], F16, tag="T")
                nc.scalar.activation(out=t, in_=kT[:, ht, :],
                                     func=mybir.ActivationFunctionType.Tanh,
                                     bias=qT[:, ht, qi:qi + 1])
                tt.append(t)
            for kb in range(KB):
                col = kb * P + qi
                for ht in range(HT):
                    nc.tensor.matmul(sc[:, col:col + 1],
                                     tt[ht][:, kb * P:(kb + 1) * P],
                                     wv_sb[:, ht:ht + 1],
                                     start=(ht == 0), stop=(ht == HT - 1))

        # --- exp (+mask) and AV ---
        po = ps_out.tile([P, V + 1], F32, tag="po")
        for kb in range(KB):
            e = persist.tile([P, Q], F32, tag=f"E{b}{kb}")
            nc.scalar.activation(out=e, in_=sc[:, kb * P:(kb + 1) * P],
                                 func=mybir.ActivationFunctionType.Exp,
                                 bias=mask_sb[:, b, kb:kb + 1])
            nc.tensor.matmul(po, e, vo[kb], start=(kb == 0), stop=(kb == KB - 1))

        r = stage.tile([P, 1], F32, tag="recip")
        nc.vector.reciprocal(out=r, in_=po[:, V:V + 1])
        ot = stage.tile([P, V], F32, tag="ot")
        nc.vector.tensor_scalar_mul(ot, po[:, :V], r)
        nc.sync.dma_start(out=out_d[b], in_=ot)


def build_nc(repeat=1):
    from contextlib import ExitStack
    nc = bacc.Bacc("TRN2", target_bir_lowering=False, debug=False,
                   num_devices=N_CORES)
    queries_d = nc.dram_tensor("queries", [B_LOC, Q, H], F32, kind="ExternalInput").ap()
    keys_d = nc.dram_tensor("keys", [B_LOC, K, H], F32, kind="ExternalInput").ap()
    values_d = nc.dram_tensor("values", [B_LOC, K, V], F32, kind="ExternalInput").ap()
    wq_d = nc.dram_tensor("Wq", [H, H], F32, kind="ExternalInput").ap()
    wk_d = nc.dram_tensor("Wk", [H, H], F32, kind="ExternalInput").ap()
    wv_d = nc.dram_tensor("Wv", [H], F32, kind="ExternalInput").ap()
    mask_d = nc.dram_tensor("mask", [B_LOC, K], F32, kind="ExternalInput").ap()
    out_d = nc.dram_tensor("out", [B_LOC, Q, V], F32, kind="ExternalOutput").ap()

    with tile.TileContext(nc) as tc, ExitStack() as ctx:
        args = (nc, tc, queries_d, keys_d, values_d, wq_d, wk_d, wv_d, mask_d, out_d, ctx)
        if repeat == 1:
            _emit(*args)
        else:
            with tc.For_i(0, repeat, 1):
                _emit(*args)
    nc.compile()
    return nc


def make_runner(nc):
    """Replicates bass2jax.run_bass_via_pjrt but jits once for reuse."""
    bass2jax.install_neuronx_cc_hook()
    partition_name = nc.partition_id_tensor.name if nc.partition_id_tensor else None
    in_names, out_names, out_avals, zero_shapes = [], [], [], []
    for alloc in nc.m.functions[0].allocations:
        if not isinstance(alloc, mybir.MemoryLocationSet):
            continue
        name = alloc.memorylocations[0].name
        if alloc.kind == "ExternalInput":
            if name != partition_name:
                in_names.append(name)
        elif alloc.kind == "ExternalOutput":
            shape = tuple(alloc.tensor_shape)
            npdt = np.dtype(mybir.dt.np(alloc.dtype))
            out_names.append(name)
            out_avals.append(jax.core.ShapedArray(shape, npdt))
            zero_shapes.append((shape, npdt))
    n_params = len(in_names)
    n_outs = len(out_avals)
    in_names_all = list(in_names) + list(out_names)
    if partition_name is not None:
        in_names_all.append(partition_name)

    def _body(*args):
        operands = list(args)
        if partition_name is not None:
            operands.append(bass2jax.partition_id_tensor())
        outs = bass2jax._bass_exec_p.bind(
            *operands,
            out_avals=tuple(out_avals),
            in_names=tuple(in_names_all),
            out_names=tuple(out_names),
            lowering_input_output_aliases=(),
            sim_require_finite=True,
            sim_require_nnan=True,
            nc=nc,
        )
        return tuple(outs)

    devices = jax.devices()[:N_CORES]
    mesh = Mesh(np.asarray(devices), ("core",))
    in_specs = (PartitionSpec("core"),) * (n_params + n_outs)
    out_specs = (PartitionSpec("core"),) * len(out_names)
    sharded = jax.jit(
        shard_map(_body, mesh=mesh, in_specs=in_specs, out_specs=out_specs,
                  check_rep=False),
        donate_argnums=tuple(range(n_params, n_params + n_outs)),
        keep_unused=True,
    )

    def run(in_maps):
        per_core = [[np.asarray(m[name]) for name in in_names] for m in in_maps]
        concat_in = [np.concatenate([per_core[c][i] for c in range(N_CORES)], axis=0)
                     for i in range(n_params)]
        concat_zeros = [np.zeros((N_CORES * s[0], *s[1:]), d) for (s, d) in zero_shapes]
        out_arrs = sharded(*concat_in, *concat_zeros)
        out_arrs = [np.asarray(a) for a in out_arrs]
        return [
            {name: out_arrs[i].reshape(N_CORES, *out_avals[i].shape)[c]
             for i, name in enumerate(out_names)}
            for c in range(N_CORES)
        ]

    return run


_RUNNERS = {}


def get_runner(repeat=1):
    if repeat not in _RUNNERS:
        _RUNNERS[repeat] = make_runner(build_nc(repeat))
    return _RUNNERS[repeat]


def make_in_maps(queries, keys, values, Wq, Wk, Wv, valid_lens):
    queries = np.asarray(queries, np.float32)
    keys = np.asarray(keys, np.float32)
    values = np.asarray(values, np.float32)
    Wq = np.asarray(Wq, np.float32)
    Wk = np.asarray(Wk, np.float32)
    Wv = np.asarray(Wv, np.float32)
    valid_lens = np.asarray(valid_lens)
    mask = np.where(np.arange(K)[None, :] < valid_lens[:, None].astype(np.int64),
                    0.0, MASK_VAL).astype(np.float32)
    in_maps = []
    for c in range(N_CORES):
        sl = slice(c * B_LOC, (c + 1) * B_LOC)
        in_maps.append({
            "queries": queries[sl], "keys": keys[sl], "values": values[sl],
            "Wq": Wq, "Wk": Wk, "Wv": Wv, "mask": mask[sl],
        })
    return in_maps


def kernel(queries, keys, values, Wq, Wk, Wv, valid_lens):
    run = get_runner(1)
    in_maps = make_in_maps(queries, keys, values, Wq, Wk, Wv, valid_lens)
    res = run(in_maps)
    return np.concatenate([res[c]["out"] for c in range(N_CORES)], axis=0)
